# revision 2
# baseline (speedup 1.0000x reference)
"""Trainium2 Bass kernel for nn_Node2Pair_bias (LayerNorm -> dual projection ->
pair outer-product -> head-mix linear).

Reference computation (B=2, L=512, D=256, DH=32, H=16, K=2, P=128):
    x   = LayerNorm(node) * gamma + beta, masked        [B, L, D]
    left  = (x @ W_left + b_left)                       [B, L, DH] -> [B,L,H,K]
    right = (x @ W_right + b_right)/sqrt(DH)            [B, L, DH] -> [B,L,H,K]
    out[b,i,j,h] = sum_k left[b,i,h,k]*right[b,j,h,k]
    out[b,i,j,p] = sum_h out[b,i,j,h]*W_out[h,p] + b_out[p]   [B, L, L, P]

Mathematical restructuring used here (c = (h,k) combined channel, 0..31):
    out[b,i,j,p] = sum_c right[b,j,c] * (left[b,i,c] * W2[c,p]) + b_out[p]
with W2[c,p] = W_out[c//2, p].  For each i we form M_i[c,p] = left[b,i,c]*W2[c,p]
with a single broadcast DVE multiply per 4-i pack: rhs [33, (q,p)=512] where
row 32 = b_out (matched by a constant-1 row 32 in rightT), then
fp32r matmul  lhsT=rightT[33, j-chunk 128]  x  rhs=M_pack[33, 512]
-> psum[j=128, (q, p)=512].  PSUM is converted f32->bf16 into SBUF staging
(2 sg-groups x 16 i's wide = 1 MiB) and DMA'd to the output shard; the host
converts back to fp32 while assembling (rel-err of bf16 rounding ~2e-3,
well inside the 2e-2 gate).

Sharding: the i axis of L is split across the 8 cores (sequence-parallel); each
core holds its [B, 64] slice of `left` inputs plus the full `right` side and
writes a [B, 64, L, P] output shard.  No cross-device communication.

LayerNorm gamma/beta are folded into the projection weights on the host
(exact algebra): W_e = gamma[:,None]*W, with an extra K=1 accumulation row
carrying beta@W * mask (mask enters as a 0/1 row since masking commutes with
the diagonal gamma scaling).
"""

import os
import sys

sys.path.insert(0, "/opt/trn_rl_repo")

import numpy as np

import concourse.bass as bass
import concourse.mybir as mybir
import concourse.tile as tile
from concourse import bacc
from concourse.bass_utils import run_bass_kernel_spmd
from concourse.masks import make_identity

F32 = mybir.dt.float32
F32R = mybir.dt.float32r
BF16 = mybir.dt.bfloat16

B, L, D = 2, 512, 256
DH, H, PAIR = 32, 16, 128
NCORES = 8
LSH = L // NCORES          # 64 i's per core per batch
LN_EPS = 1e-5

_COMPILED = None  # (nc, input_names)


def _build_program():
    nc = bacc.Bacc("TRN2", target_bir_lowering=False, debug=False,
                   num_devices=NCORES)

    # ---------------- DRAM parameters ----------------
    def din(name, shape):
        return nc.dram_tensor(name, list(shape), F32, kind="ExternalInput").ap()

    node_full = din("node_full", (B * L, D))        # all rows, (b,l) major
    node_shard = din("node_shard", (B * LSH, D))    # this core's i rows, (b,i)
    mask_col_full = din("mask_col_full", (128, B * L // 128))  # [:, t] = tile t
    mask_col_shard = din("mask_col_shard", (128, 1))
    mask_row_full = din("mask_row_full", (B, L))    # 0/1 rows per batch
    mask_row_shard = din("mask_row_shard", (1, B * LSH))
    w_left_e = din("w_left_e", (D + 1, DH))         # rows: gamma*W_l; last: beta@W_l
    w_right_e = din("w_right_e", (D + 1, DH))       # scaled by 1/sqrt(DH)
    b_left_col = din("b_left_col", (DH, 1))
    b_right_col = din("b_right_col", (DH, 1))       # scaled by 1/sqrt(DH)
    w2 = din("w2", (DH + 1, PAIR))      # W_out rows repeated x2, then b_out row

    # Permuted output layout: [b, jc, q2, j, s, i16, p] (bf16) — each staging
    # buffer lands as one fully contiguous 1 MiB stream (8 KiB per partition
    # run).  sg = q2*2 + s.  The host un-permutes + upcasts while assembling
    # the full output.
    out = nc.dram_tensor("out", [B, 4, 2, 128, 2, 16, PAIR], BF16,
                         kind="ExternalOutput").ap()

    NT_FULL = B * L // 128   # 8 LayerNorm tiles for the full sequence

    with tile.TileContext(nc) as tc:
        with (
            tc.tile_pool(name="singles", bufs=1) as singles,
            tc.tile_pool(name="xpool", bufs=9) as xpool,
            tc.tile_pool(name="stats", bufs=4) as stats,
            tc.tile_pool(name="persist", bufs=1) as persist,
            tc.tile_pool(name="mp", bufs=16) as mp_pool,
            tc.tile_pool(name="stag", bufs=8) as stag_pool,
            tc.tile_pool(name="ps_tp", bufs=1, space="PSUM") as ps_tp,
            tc.tile_pool(name="ps_proj", bufs=1, space="PSUM") as ps_proj,
            tc.tile_pool(name="ps_big", bufs=5, space="PSUM") as ps_big,
        ):
            # ---------------- constants ----------------
            ident = singles.tile([128, 128], F32, tag="ident")
            make_identity(nc, ident)
            eps_t = singles.tile([128, 1], F32, tag="eps")
            nc.vector.memset(eps_t, LN_EPS)

            # hot-path loads on sync (HWDGE) in dependency-critical order;
            # right-side constants via gpsimd (SWDGE) in parallel
            xs = xpool.tile([128, D], F32, tag="x", name="xs")
            nc.sync.dma_start(out=xs, in_=node_shard[:, :])
            mcs_sb = singles.tile([128, 1], F32, tag="mcs")
            nc.sync.dma_start(out=mcs_sb, in_=mask_col_shard[:, :])
            xf_tiles = [None] * NT_FULL
            for t in range(4):
                xf = xpool.tile([128, D], F32, tag="x", name=f"xf{t}")
                nc.sync.dma_start(out=xf,
                                  in_=node_full[t * 128:(t + 1) * 128, :])
                xf_tiles[t] = xf
            mcf_sb = singles.tile([128, NT_FULL], F32, tag="mcf")
            nc.sync.dma_start(out=mcf_sb, in_=mask_col_full[:, :])
            wl_sb = [singles.tile([128, DH], F32, tag=f"wl{dc}", name=f"wl{dc}")
                     for dc in range(2)]
            wl_row = singles.tile([1, DH], F32, tag="wlrow")
            for dc in range(2):
                nc.sync.dma_start(out=wl_sb[dc],
                                  in_=w_left_e[dc * 128:(dc + 1) * 128, :])
            nc.sync.dma_start(out=wl_row, in_=w_left_e[D:D + 1, :])
            bl_sb = singles.tile([DH, 1], F32, tag="bl")
            nc.sync.dma_start(out=bl_sb, in_=b_left_col[:, :])
            w2_sb = singles.tile([DH + 1, PAIR], F32, tag="w2")
            nc.sync.dma_start(out=w2_sb, in_=w2[:, :])
            for t in range(4, NT_FULL):
                xf = xpool.tile([128, D], F32, tag="x", name=f"xf{t}")
                nc.sync.dma_start(out=xf,
                                  in_=node_full[t * 128:(t + 1) * 128, :])
                xf_tiles[t] = xf

            wr_sb = [singles.tile([128, DH], F32, tag=f"wr{dc}", name=f"wr{dc}")
                     for dc in range(2)]
            wr_row = singles.tile([1, DH], F32, tag="wrrow")
            for dc in range(2):
                nc.gpsimd.dma_start(out=wr_sb[dc],
                                    in_=w_right_e[dc * 128:(dc + 1) * 128, :])
            nc.gpsimd.dma_start(out=wr_row, in_=w_right_e[D:D + 1, :])
            br_sb = singles.tile([DH, 1], F32, tag="br")
            nc.gpsimd.dma_start(out=br_sb, in_=b_right_col[:, :])
            mrf_sb = [singles.tile([1, L], F32, tag=f"mrf{b}", name=f"mrf{b}")
                      for b in range(B)]
            for b in range(B):
                nc.gpsimd.dma_start(out=mrf_sb[b],
                                    in_=mask_row_full[b:b + 1, :])
            mrs_sb = singles.tile([1, B * LSH], F32, tag="mrs")
            nc.gpsimd.dma_start(out=mrs_sb, in_=mask_row_shard[:, :])
            ones_row = singles.tile([1, L], F32, tag="ones")
            nc.vector.memset(ones_row, 1.0)

            # ---------------- LayerNorm helper ----------------
            def layernorm_masked(x_t, mask_col_ap):
                """x_t [128, D] in place -> (x - mu) * rsqrt(var+eps) * mask."""
                st = stats.tile([128, 6], F32, tag="st")
                nc.vector.bn_stats(out=st, in_=x_t)
                mv = stats.tile([128, 2], F32, tag="mv")
                nc.vector.bn_aggr(out=mv, in_=st)
                sd = stats.tile([128, 1], F32, tag="sd")
                nc.scalar.activation(out=sd, in_=mv[:, 1:2],
                                     func=mybir.ActivationFunctionType.Sqrt,
                                     bias=eps_t, scale=1.0)
                rs = stats.tile([128, 1], F32, tag="rs")
                nc.vector.reciprocal(out=rs, in_=sd)
                rsm = stats.tile([128, 1], F32, tag="rsm")
                nc.vector.tensor_mul(out=rsm, in0=rs, in1=mask_col_ap)
                nc.vector.tensor_scalar(out=x_t, in0=x_t,
                                        scalar1=mv[:, 0:1], scalar2=rsm,
                                        op0=mybir.AluOpType.subtract,
                                        op1=mybir.AluOpType.mult)

            # ---------------- shard path: leftT_all [DH, B*LSH] ----------------
            layernorm_masked(xs, mcs_sb[:, 0:1])

            xsT = [persist.tile([128, B * LSH], F32, tag=f"xsT{dc}", name=f"xsT{dc}")
                   for dc in range(2)]
            for dc in range(2):
                pt = ps_tp.tile([128, 128], F32, tag="tp")
                nc.tensor.transpose(pt, xs[:, dc * 128:(dc + 1) * 128], ident)
                nc.scalar.copy(out=xsT[dc], in_=pt)

            ps_l = ps_proj.tile([DH, L], F32, tag="pr", name="ps_l")
            ps_l = ps_l[:, 0:B * LSH]
            for dc in range(2):
                nc.tensor.matmul(ps_l, wl_sb[dc], xsT[dc],
                                 start=(dc == 0), stop=False)
            nc.tensor.matmul(ps_l, wl_row, mrs_sb, start=False, stop=True)
            leftT = persist.tile([DH + 1, B * LSH], F32, tag="leftT")
            nc.vector.tensor_scalar_add(out=leftT[0:DH, :], in0=ps_l,
                                        scalar1=bl_sb)
            nc.vector.memset(leftT[DH:DH + 1, :], 1.0)

            # ---------------- full path: rightT_r[b] [33, L] fp32r ----------------
            rightT = [persist.tile([DH + 1, L], F32R, tag=f"rt{b}", name=f"rt{b}")
                      for b in range(B)]
            xT = [[persist.tile([128, L], F32, tag=f"xT{b}_{dc}", name=f"xT{b}_{dc}")
                   for dc in range(2)] for b in range(B)]
            for b in range(B):
                for lc in range(4):
                    t = b * 4 + lc
                    xf = xf_tiles[t]
                    layernorm_masked(xf, mcf_sb[:, t:t + 1])
                    for dc in range(2):
                        pt = ps_tp.tile([128, 128], F32, tag="tp")
                        nc.tensor.transpose(pt, xf[:, dc * 128:(dc + 1) * 128],
                                            ident)
                        eng = nc.vector if (lc + dc) % 2 == 0 else nc.scalar
                        if eng is nc.vector:
                            nc.vector.tensor_copy(
                                out=xT[b][dc][:, lc * 128:(lc + 1) * 128],
                                in_=pt)
                        else:
                            nc.scalar.copy(
                                out=xT[b][dc][:, lc * 128:(lc + 1) * 128],
                                in_=pt)

                ps_r = ps_proj.tile([DH, L], F32, tag="pr")
                for jc in range(4):
                    jsl = slice(jc * 128, (jc + 1) * 128)
                    for dc in range(2):
                        nc.tensor.matmul(ps_r[:, jsl], wr_sb[dc],
                                         xT[b][dc][:, jsl],
                                         start=(dc == 0), stop=False)
                    nc.tensor.matmul(ps_r[:, jsl], wr_row, mrf_sb[b][:, jsl],
                                     start=False, stop=True)
                    nc.vector.tensor_scalar_add(out=rightT[b][0:DH, jsl],
                                                in0=ps_r[:, jsl],
                                                scalar1=br_sb)
                nc.vector.tensor_copy(out=rightT[b][DH:DH + 1, :],
                                      in_=ones_row)

            # ---------------- main pair loop ----------------
            # Round = (b, q2): 8 M_packs (one broadcast DVE/GpSimd multiply
            # each), then per j-chunk 8 matmuls (sg pair) whose psums are
            # converted to bf16 into a 1 MiB staging tile, DMA'd as soon as
            # the tile completes.
            def build_mp(b, sg, il):
                """[33, 512] pack: mp[c, q*128+p] = left[b, i(sg,il,q), c] * w2[c, p]."""
                mp = mp_pool.tile([DH + 1, 512], F32R, tag="mp",
                                  name=f"mp{b}_{sg}_{il}")
                col = b * LSH + (sg * 4 + il) * 4
                lsrc = leftT[:, col:col + 4].unsqueeze(-1).to_broadcast(
                    [DH + 1, 4, PAIR])
                wsrc = w2_sb[:, :].unsqueeze(1).to_broadcast(
                    [DH + 1, 4, PAIR])
                dst = mp[:, :].rearrange("c (q p) -> c q p", p=PAIR)
                if il % 2 == 0:
                    nc.vector.tensor_mul(out=dst, in0=wsrc, in1=lsrc)
                else:
                    nc.gpsimd.tensor_mul(out=dst, in0=wsrc, in1=lsrc)
                return mp

            for b in range(B):
                for q2 in range(2):
                    mps = [[build_mp(b, q2 * 2 + s, il) for il in range(4)]
                           for s in range(2)]
                    for jc in range(4):
                        lhsT = rightT[b][:, jc * 128:(jc + 1) * 128]
                        stg = stag_pool.tile([128, 2 * 16 * PAIR], BF16,
                                             tag="stag")
                        nco = 0
                        for s in range(2):
                            for il in range(4):
                                pb = ps_big.tile([128, 512], F32, tag="big")
                                nc.tensor.matmul(pb, lhsT, mps[s][il],
                                                 start=True, stop=True)
                                dst = stg[:, (s * 16 + il * 4) * PAIR:
                                          (s * 16 + il * 4 + 4) * PAIR]
                                # DVE copies are ~2.2x faster than ACT ones;
                                # give ACT 3 of 8 per j-chunk.
                                if nco in (1, 4, 6):
                                    nc.scalar.copy(out=dst, in_=pb)
                                else:
                                    nc.vector.tensor_copy(out=dst, in_=pb)
                                nco += 1
                        dst_ap = out[b, jc, q2, :, :, :, :]
                        src_ap = stg[:, :].rearrange("j (s i p) -> j s i p",
                                                     s=2, p=PAIR)
                        deng = nc.sync if jc % 2 == 0 else nc.gpsimd
                        deng.dma_start(out=dst_ap, in_=src_ap)

    nc.compile()
    names = ["node_full", "node_shard", "mask_col_full", "mask_col_shard",
             "mask_row_full", "mask_row_shard", "w_left_e", "w_right_e",
             "b_left_col", "b_right_col", "w2"]
    return nc, names


def _prepare_in_maps(node, mask, ln_gamma, ln_beta, W_left, b_left, W_right,
                     b_right, W_out, b_out):
    f = np.float32
    node = np.ascontiguousarray(np.asarray(node, dtype=f))        # [B, L, D]
    mask_f = np.asarray(mask).astype(f)                           # [B, L]
    gamma = np.asarray(ln_gamma, dtype=f)
    beta = np.asarray(ln_beta, dtype=f)
    W_l = np.asarray(W_left, dtype=f)
    W_r = np.asarray(W_right, dtype=f)
    b_l = np.asarray(b_left, dtype=f)
    b_r = np.asarray(b_right, dtype=f)
    W_o = np.asarray(W_out, dtype=f)
    b_o = np.asarray(b_out, dtype=f)

    s = 1.0 / np.sqrt(np.float32(DH))
    w_left_e = np.concatenate([gamma[:, None] * W_l, (beta @ W_l)[None, :]], 0)
    w_right_e = np.concatenate([gamma[:, None] * W_r, (beta @ W_r)[None, :]],
                               0) * s
    w2 = np.concatenate([np.repeat(W_o, 2, axis=0), b_o[None, :]], 0)

    node_flat = node.reshape(B * L, D)
    mask_col_full = np.ascontiguousarray(mask_f.reshape(-1, 128).T)  # [128, 8]
    mask_row_full = np.ascontiguousarray(mask_f)                     # [B, L]

    common = {
        "node_full": node_flat,
        "mask_col_full": mask_col_full,
        "mask_row_full": mask_row_full,
        "w_left_e": np.ascontiguousarray(w_left_e),
        "w_right_e": np.ascontiguousarray(w_right_e),
        "b_left_col": np.ascontiguousarray(b_l[:, None]),
        "b_right_col": np.ascontiguousarray(b_r[:, None] * s),
        "w2": np.ascontiguousarray(w2),
    }

    in_maps = []
    for c in range(NCORES):
        sl = slice(c * LSH, (c + 1) * LSH)
        shard = np.ascontiguousarray(node[:, sl, :].reshape(B * LSH, D))
        msk = mask_f[:, sl]                                       # [B, LSH]
        m = dict(common)
        m["node_shard"] = shard
        m["mask_col_shard"] = np.ascontiguousarray(msk.reshape(-1)[:, None])
        m["mask_row_shard"] = np.ascontiguousarray(msk.reshape(1, -1))
        in_maps.append(m)
    return in_maps


def kernel(**inputs):
    global _COMPILED
    if _COMPILED is None:
        _COMPILED = _build_program()
    nc, names = _COMPILED
    in_maps = _prepare_in_maps(**inputs)
    res = run_bass_kernel_spmd(nc, in_maps, core_ids=list(range(NCORES)))
    full = np.empty((B, L, L, PAIR), np.float32)
    for c in range(NCORES):
        dev = res.results[c]["out"]   # [b, jc, q2, j, s, i16, p] bf16
        full[:, c * LSH:(c + 1) * LSH] = (
            dev.transpose(0, 2, 4, 5, 1, 3, 6)
               .reshape(B, LSH, L, PAIR).astype(np.float32))
    return full


if __name__ == "__main__":
    # self-test with NON-trivial gamma/beta/mask against a numpy reference
    rng = np.random.default_rng(1)
    mask = np.ones((B, L), dtype=bool)
    mask[0, 500:] = False        # exercise the mask path
    mask[1, :3] = False
    inputs = {
        "node": rng.standard_normal((B, L, D)).astype(np.float32),
        "mask": mask,
        "ln_gamma": (1.0 + 0.1 * rng.standard_normal(D)).astype(np.float32),
        "ln_beta": (0.1 * rng.standard_normal(D)).astype(np.float32),
        "W_left": (rng.standard_normal((D, DH)) / np.sqrt(D)).astype(np.float32),
        "b_left": (0.1 * rng.standard_normal(DH)).astype(np.float32),
        "W_right": (rng.standard_normal((D, DH)) / np.sqrt(D)).astype(np.float32),
        "b_right": (0.1 * rng.standard_normal(DH)).astype(np.float32),
        "W_out": (rng.standard_normal((H, PAIR)) / np.sqrt(H)).astype(np.float32),
        "b_out": (0.1 * rng.standard_normal(PAIR)).astype(np.float32),
    }

    def np_reference(node, mask, ln_gamma, ln_beta, W_left, b_left, W_right,
                     b_right, W_out, b_out):
        node = node.astype(np.float64)
        mu = node.mean(-1, keepdims=True)
        var = ((node - mu) ** 2).mean(-1, keepdims=True)
        x = (node - mu) / np.sqrt(var + LN_EPS) * ln_gamma + ln_beta
        x = x * mask[..., None]
        left = (x @ W_left + b_left).reshape(B, L, H, -1)
        right = ((x @ W_right + b_right) / np.sqrt(DH)).reshape(B, L, H, -1)
        o = np.einsum("bihk,bjhk->bijh", left, right)
        return np.einsum("bijh,hp->bijp", o, W_out) + b_out

    got = kernel(**inputs)
    exp = np_reference(**inputs)
    rel = np.abs(got - exp).max() / np.abs(exp).max()
    print("general-path rel err:", rel)
    assert rel < 1e-2, rel
    print("OK", got.shape, got.dtype)


# revision 6
# speedup vs baseline: 1.0996x; 1.0996x over previous
"""Trainium2 Bass kernel for nn_Node2Pair_bias (LayerNorm -> dual projection ->
pair outer-product -> head-mix linear).

Reference computation (B=2, L=512, D=256, DH=32, H=16, K=2, P=128):
    x   = LayerNorm(node) * gamma + beta, masked        [B, L, D]
    left  = (x @ W_left + b_left)                       [B, L, DH] -> [B,L,H,K]
    right = (x @ W_right + b_right)/sqrt(DH)            [B, L, DH] -> [B,L,H,K]
    out[b,i,j,h] = sum_k left[b,i,h,k]*right[b,j,h,k]
    out[b,i,j,p] = sum_h out[b,i,j,h]*W_out[h,p] + b_out[p]   [B, L, L, P]

Mathematical restructuring (c = (h,k) combined channel, 0..31):
    out[b,i,j,p] = sum_c right[b,j,c] * (left[b,i,c] * W2[c,p]) + b_out[p]
with W2[c,p] = W_out[c//2, p].  M-packs M[c, (q,p)] = left[b,i_q,c]*W2[c,p]
for 4 i's are built in one broadcast multiply each; the pair matmul is
bf16 x bf16 -> fp32 PSUM:  lhsT=rightT[33, j-chunk 128] x rhs=M_pack[33, 512].
Row 32 of rightT is constant 1 and row 32 of the M-pack is b_out, which adds
the bias inside the same matmul.

PSUM is drained in [128, 1024] bank pairs with an f32->bf16 converting copy
(DVE/ACT alternating) into 1 MiB staging tiles, DMA'd per j-chunk.  The host
converts bf16 back to fp32 while assembling (bf16 + bf16-matmul rounding is
~6e-3 max-rel, inside the 2e-2 gate).

Sharding: the i axis of L is split across the 8 cores (sequence-parallel);
each core holds its [B, 64] slice of `left` plus the full `right` side and
writes a [B, 64, L, P] output shard.  No cross-device communication.

LayerNorm gamma/beta are folded into the projection weights on the host
(exact algebra): W_e = gamma[:,None]*W, with an extra K=1 accumulation row
carrying beta@W * mask.
"""

import os
import sys

sys.path.insert(0, "/opt/trn_rl_repo")

import numpy as np

import concourse.bass as bass
import concourse.mybir as mybir
import concourse.tile as tile
from concourse import bacc
from concourse.bass_utils import run_bass_kernel_spmd
from concourse.masks import make_identity

F32 = mybir.dt.float32
F32R = mybir.dt.float32r
BF16 = mybir.dt.bfloat16

B, L, D = 2, 512, 256
DH, H, PAIR = 32, 16, 128
NCORES = 8
LSH = L // NCORES          # 64 i's per core per batch
LN_EPS = 1e-5

# packed-constant column map (one [128, NCONST] f32 tensor, one DMA)
COL_WL = (0, 32)           # [128, 32] x2: gamma*W_l rows 0-127 / 128-255
COL_WR = (64, 96)
COL_W2 = 128               # [33, 128] (cast to bf16 on chip)
COL_MCF = 256              # [128, 8]
COL_MCS = 264              # [128, 1]
COL_BL = 265               # [32, 1]
COL_BR = 266               # [32, 1]
COL_WLR = 267              # [1, 32]  row 256 of w_left_e
COL_WRR = 299              # [1, 32]
COL_MRS = 331              # [1, 128]
COL_MRF = (459, 971)       # [1, 512] x2
NCONST = 1483

_COMPILED = None  # (nc, input_names)


def _build_program():
    nc = bacc.Bacc("TRN2", target_bir_lowering=False, debug=False,
                   num_devices=NCORES)

    node_full = nc.dram_tensor("node_full", [B * L, D], F32,
                               kind="ExternalInput").ap()
    node_shard = nc.dram_tensor("node_shard", [B * LSH, D], F32,
                                kind="ExternalInput").ap()
    consts = nc.dram_tensor("consts", [128, NCONST], F32,
                            kind="ExternalInput").ap()

    # Permuted output layout: [b, jc, q2, j, s, i16, p] (bf16) — each staging
    # buffer lands as one fully contiguous 1 MiB stream (8 KiB per partition
    # run).  sg = q2*2 + s; i_local = sg*16 + i16.  The host un-permutes +
    # upcasts while assembling the full output.
    out = nc.dram_tensor("out", [B, 4, 2, 128, 2, 16, PAIR], BF16,
                         kind="ExternalOutput").ap()

    with tile.TileContext(nc) as tc:
        with (
            tc.tile_pool(name="singles", bufs=1) as singles,
            tc.tile_pool(name="xpool", bufs=9) as xpool,
            tc.tile_pool(name="stats", bufs=4) as stats,
            tc.tile_pool(name="persist", bufs=1) as persist,
            tc.tile_pool(name="mp", bufs=16) as mp_pool,
            tc.tile_pool(name="stag", bufs=8) as stag_pool,
            tc.tile_pool(name="ps_tp", bufs=2, space="PSUM") as ps_tp,
            tc.tile_pool(name="ps_proj", bufs=1, space="PSUM") as ps_proj,
            tc.tile_pool(name="ps_big", bufs=2, space="PSUM") as ps_big,
        ):
            # ---------------- input loads (3 queues in parallel) ------------
            xs = xpool.tile([128, D], F32, tag="x", name="xs")
            nc.sync.dma_start(out=xs, in_=node_shard[:, :])
            cst = singles.tile([128, NCONST], F32, tag="cst")
            nc.scalar.dma_start(out=cst, in_=consts[:, :])
            xf_tiles = []
            qs = [nc.sync, nc.scalar, nc.gpsimd]
            for t in range(8):
                xf = xpool.tile([128, D], F32, tag="x", name=f"xf{t}")
                qs[[0, 0, 2, 2, 1, 1, 2, 0][t]].dma_start(
                    out=xf, in_=node_full[t * 128:(t + 1) * 128, :])
                xf_tiles.append(xf)

            # ---------------- constants / views ----------------
            ident = singles.tile([128, 128], F32, tag="ident")
            make_identity(nc, ident)
            eps_t = singles.tile([128, 1], F32, tag="eps")
            nc.vector.memset(eps_t, LN_EPS)

            wl_sb = [cst[:, COL_WL[dc]:COL_WL[dc] + DH] for dc in range(2)]
            wr_sb = [cst[:, COL_WR[dc]:COL_WR[dc] + DH] for dc in range(2)]
            wl_row = cst[0:1, COL_WLR:COL_WLR + DH]
            wr_row = cst[0:1, COL_WRR:COL_WRR + DH]
            bl_sb = cst[0:DH, COL_BL:COL_BL + 1]
            br_sb = cst[0:DH, COL_BR:COL_BR + 1]
            mcf_sb = cst[:, COL_MCF:COL_MCF + 8]
            mcs_sb = cst[:, COL_MCS:COL_MCS + 1]
            mrs_sb = cst[0:1, COL_MRS:COL_MRS + B * LSH]
            mrf_sb = [cst[0:1, COL_MRF[b]:COL_MRF[b] + L] for b in range(B)]

            w2bf = singles.tile([DH + 1, PAIR], BF16, tag="w2bf")
            nc.scalar.copy(out=w2bf, in_=cst[0:DH + 1, COL_W2:COL_W2 + PAIR])

            # ---------------- LayerNorm helper ----------------
            def layernorm_masked(x_t, mask_col_ap):
                """x_t [128, D] in place -> (x - mu) * rsqrt(var+eps) * mask."""
                st = stats.tile([128, 6], F32, tag="st")
                nc.vector.bn_stats(out=st, in_=x_t)
                mv = stats.tile([128, 2], F32, tag="mv")
                nc.vector.bn_aggr(out=mv, in_=st)
                sd = stats.tile([128, 1], F32, tag="sd")
                nc.scalar.activation(out=sd, in_=mv[:, 1:2],
                                     func=mybir.ActivationFunctionType.Sqrt,
                                     bias=eps_t, scale=1.0)
                rs = stats.tile([128, 1], F32, tag="rs")
                nc.vector.reciprocal(out=rs, in_=sd)
                rsm = stats.tile([128, 1], F32, tag="rsm")
                nc.vector.tensor_mul(out=rsm, in0=rs, in1=mask_col_ap)
                nc.vector.tensor_scalar(out=x_t, in0=x_t,
                                        scalar1=mv[:, 0:1], scalar2=rsm,
                                        op0=mybir.AluOpType.subtract,
                                        op1=mybir.AluOpType.mult)

            # ---------------- shard path: leftT_all [33, B*LSH] bf16 --------
            layernorm_masked(xs, mcs_sb)

            xsT = [persist.tile([128, B * LSH], F32, tag=f"xsT{dc}",
                                name=f"xsT{dc}") for dc in range(2)]
            for dc in range(2):
                pt = ps_tp.tile([128, 128], F32, tag="tp")
                nc.tensor.transpose(pt, xs[:, dc * 128:(dc + 1) * 128], ident)
                nc.scalar.copy(out=xsT[dc], in_=pt)

            ps_l = ps_proj.tile([DH, L], F32, tag="pr", name="ps_l")
            ps_l = ps_l[:, 0:B * LSH]
            for dc in range(2):
                nc.tensor.matmul(ps_l, wl_sb[dc], xsT[dc],
                                 start=(dc == 0), stop=False)
            nc.tensor.matmul(ps_l, wl_row, mrs_sb, start=False, stop=True)
            leftT = persist.tile([DH + 1, B * LSH], BF16, tag="leftT")
            nc.vector.tensor_scalar_add(out=leftT[0:DH, :], in0=ps_l,
                                        scalar1=bl_sb)
            nc.vector.memset(leftT[DH:DH + 1, :], 1.0)

            # ---------------- full path (per batch): rightT[b] [33, L] bf16 -
            rightT = [persist.tile([DH + 1, L], BF16, tag=f"rt{b}",
                                   name=f"rt{b}") for b in range(B)]
            xT = [[persist.tile([128, L], F32, tag=f"xT{b}_{dc}",
                                name=f"xT{b}_{dc}") for dc in range(2)]
                  for b in range(B)]

            def build_right(b):
                ps_r = ps_proj.tile([DH, L], F32, tag="pr", name=f"ps_r{b}")
                for lc in range(4):
                    t = b * 4 + lc
                    xf = xf_tiles[t]
                    layernorm_masked(xf, mcf_sb[:, t:t + 1])
                    for dc in range(2):
                        pt = ps_tp.tile([128, 128], F32, tag="tp")
                        nc.tensor.transpose(pt, xf[:, dc * 128:(dc + 1) * 128],
                                            ident)
                        if (lc + dc) % 2 == 0:
                            nc.vector.tensor_copy(
                                out=xT[b][dc][:, lc * 128:(lc + 1) * 128],
                                in_=pt)
                        else:
                            nc.scalar.copy(
                                out=xT[b][dc][:, lc * 128:(lc + 1) * 128],
                                in_=pt)
                    # project this j-chunk as soon as its columns exist
                    jc = lc
                    jsl = slice(jc * 128, (jc + 1) * 128)
                    for dc in range(2):
                        nc.tensor.matmul(ps_r[:, jsl], wr_sb[dc],
                                         xT[b][dc][:, jsl],
                                         start=(dc == 0), stop=False)
                    nc.tensor.matmul(ps_r[:, jsl], wr_row, mrf_sb[b][:, jsl],
                                     start=False, stop=True)
                    nc.vector.tensor_scalar_add(out=rightT[b][0:DH, jsl],
                                                in0=ps_r[:, jsl],
                                                scalar1=br_sb)
                nc.vector.memset(rightT[b][DH:DH + 1, :], 1.0)

            # ---------------- main pair loop ----------------
            def build_mp(b, sg, il):
                """[33, 512] bf16: mp[c, q*128+p] = left[b, i(sg,il,q), c] * w2[c, p]."""
                mp = mp_pool.tile([DH + 1, 512], BF16, tag="mp",
                                  name=f"mp{b}_{sg}_{il}")
                col = b * LSH + (sg * 4 + il) * 4
                lsrc = leftT[:, col:col + 4].unsqueeze(-1).to_broadcast(
                    [DH + 1, 4, PAIR])
                wsrc = w2bf[:, :].unsqueeze(1).to_broadcast([DH + 1, 4, PAIR])
                dst = mp[:, :].rearrange("c (q p) -> c q p", p=PAIR)
                nc.gpsimd.tensor_mul(out=dst, in0=wsrc, in1=lsrc)
                return mp

            def pair_rounds(b):
                for q2 in range(2):
                    mps = [[build_mp(b, q2 * 2 + s, il) for il in range(4)]
                           for s in range(2)]
                    for jc in range(4):
                        lhsT = rightT[b][:, jc * 128:(jc + 1) * 128]
                        stg = stag_pool.tile([128, 2 * 16 * PAIR], BF16,
                                             tag="stag")
                        for s in range(2):
                            for ilp in range(2):
                                pb = ps_big.tile([128, 1024], F32, tag="big")
                                nc.tensor.matmul(pb[:, 0:512], lhsT,
                                                 mps[s][2 * ilp],
                                                 start=True, stop=True)
                                nc.tensor.matmul(pb[:, 512:1024], lhsT,
                                                 mps[s][2 * ilp + 1],
                                                 start=True, stop=True)
                                dst = stg[:, (s * 16 + ilp * 8) * PAIR:
                                          (s * 16 + ilp * 8 + 8) * PAIR]
                                if (s + ilp) % 2 == 0:
                                    nc.vector.tensor_copy(out=dst, in_=pb)
                                else:
                                    nc.scalar.copy(out=dst, in_=pb)
                        dst_ap = out[b, jc, q2, :, :, :, :]
                        src_ap = stg[:, :].rearrange("j (s i p) -> j s i p",
                                                     s=2, p=PAIR)
                        deng = nc.sync if jc % 2 == 0 else nc.gpsimd
                        deng.dma_start(out=dst_ap, in_=src_ap)

            build_right(0)
            pair_rounds(0)
            build_right(1)
            pair_rounds(1)

    nc.compile()
    names = ["node_full", "node_shard", "consts"]
    return nc, names


def _prepare_in_maps(node, mask, ln_gamma, ln_beta, W_left, b_left, W_right,
                     b_right, W_out, b_out):
    f = np.float32
    node = np.ascontiguousarray(np.asarray(node, dtype=f))        # [B, L, D]
    mask_f = np.asarray(mask).astype(f)                           # [B, L]
    gamma = np.asarray(ln_gamma, dtype=f)
    beta = np.asarray(ln_beta, dtype=f)
    W_l = np.asarray(W_left, dtype=f)
    W_r = np.asarray(W_right, dtype=f)
    b_l = np.asarray(b_left, dtype=f)
    b_r = np.asarray(b_right, dtype=f)
    W_o = np.asarray(W_out, dtype=f)
    b_o = np.asarray(b_out, dtype=f)

    s = 1.0 / np.sqrt(np.float32(DH))
    w_left_e = np.concatenate([gamma[:, None] * W_l, (beta @ W_l)[None, :]], 0)
    w_right_e = np.concatenate([gamma[:, None] * W_r, (beta @ W_r)[None, :]],
                               0) * s
    w2 = np.concatenate([np.repeat(W_o, 2, axis=0), b_o[None, :]], 0)

    common_cst = np.zeros((128, NCONST), f)
    for dc in range(2):
        common_cst[:, COL_WL[dc]:COL_WL[dc] + DH] = \
            w_left_e[dc * 128:(dc + 1) * 128]
        common_cst[:, COL_WR[dc]:COL_WR[dc] + DH] = \
            w_right_e[dc * 128:(dc + 1) * 128]
    common_cst[0:DH + 1, COL_W2:COL_W2 + PAIR] = w2
    common_cst[:, COL_MCF:COL_MCF + 8] = mask_f.reshape(-1, 128).T
    common_cst[0:DH, COL_BL] = b_l
    common_cst[0:DH, COL_BR] = b_r * s
    common_cst[0, COL_WLR:COL_WLR + DH] = w_left_e[D]
    common_cst[0, COL_WRR:COL_WRR + DH] = w_right_e[D]
    for b in range(B):
        common_cst[0, COL_MRF[b]:COL_MRF[b] + L] = mask_f[b]

    node_flat = node.reshape(B * L, D)

    in_maps = []
    for c in range(NCORES):
        sl = slice(c * LSH, (c + 1) * LSH)
        shard = np.ascontiguousarray(node[:, sl, :].reshape(B * LSH, D))
        msk = mask_f[:, sl]                                       # [B, LSH]
        cstc = common_cst.copy()
        cstc[:, COL_MCS] = msk.reshape(-1)
        cstc[0, COL_MRS:COL_MRS + B * LSH] = msk.reshape(-1)
        in_maps.append({
            "node_full": node_flat,
            "node_shard": shard,
            "consts": cstc,
        })
    return in_maps


def kernel(**inputs):
    global _COMPILED
    if _COMPILED is None:
        _COMPILED = _build_program()
    nc, names = _COMPILED
    in_maps = _prepare_in_maps(**inputs)
    res = run_bass_kernel_spmd(nc, in_maps, core_ids=list(range(NCORES)))
    full = np.empty((B, L, L, PAIR), np.float32)
    for c in range(NCORES):
        dev = res.results[c]["out"]   # [b, jc, q2, j, s, i16, p] bf16
        full[:, c * LSH:(c + 1) * LSH] = (
            dev.transpose(0, 2, 4, 5, 1, 3, 6)
               .reshape(B, LSH, L, PAIR).astype(np.float32))
    return full


if __name__ == "__main__":
    # self-test with NON-trivial gamma/beta/mask against a numpy reference
    rng = np.random.default_rng(1)
    mask = np.ones((B, L), dtype=bool)
    mask[0, 500:] = False        # exercise the mask path
    mask[1, :3] = False
    inputs = {
        "node": rng.standard_normal((B, L, D)).astype(np.float32),
        "mask": mask,
        "ln_gamma": (1.0 + 0.1 * rng.standard_normal(D)).astype(np.float32),
        "ln_beta": (0.1 * rng.standard_normal(D)).astype(np.float32),
        "W_left": (rng.standard_normal((D, DH)) / np.sqrt(D)).astype(np.float32),
        "b_left": (0.1 * rng.standard_normal(DH)).astype(np.float32),
        "W_right": (rng.standard_normal((D, DH)) / np.sqrt(D)).astype(np.float32),
        "b_right": (0.1 * rng.standard_normal(DH)).astype(np.float32),
        "W_out": (rng.standard_normal((H, PAIR)) / np.sqrt(H)).astype(np.float32),
        "b_out": (0.1 * rng.standard_normal(PAIR)).astype(np.float32),
    }

    def np_reference(node, mask, ln_gamma, ln_beta, W_left, b_left, W_right,
                     b_right, W_out, b_out):
        node = node.astype(np.float64)
        mu = node.mean(-1, keepdims=True)
        var = ((node - mu) ** 2).mean(-1, keepdims=True)
        x = (node - mu) / np.sqrt(var + LN_EPS) * ln_gamma + ln_beta
        x = x * mask[..., None]
        left = (x @ W_left + b_left).reshape(B, L, H, -1)
        right = ((x @ W_right + b_right) / np.sqrt(DH)).reshape(B, L, H, -1)
        o = np.einsum("bihk,bjhk->bijh", left, right)
        return np.einsum("bijh,hp->bijp", o, W_out) + b_out

    got = kernel(**inputs)
    exp = np_reference(**inputs)
    rel = np.abs(got - exp).max() / np.abs(exp).max()
    print("general-path rel err:", rel)
    assert rel < 1.5e-2, rel
    print("OK", got.shape, got.dtype)


# revision 12
# speedup vs baseline: 1.1223x; 1.0207x over previous
"""Trainium2 Bass kernel for nn_Node2Pair_bias (LayerNorm -> dual projection ->
pair outer-product -> head-mix linear).

Reference computation (B=2, L=512, D=256, DH=32, H=16, K=2, P=128):
    x   = LayerNorm(node) * gamma + beta, masked        [B, L, D]
    left  = (x @ W_left + b_left)                       [B, L, DH] -> [B,L,H,K]
    right = (x @ W_right + b_right)/sqrt(DH)            [B, L, DH] -> [B,L,H,K]
    out[b,i,j,h] = sum_k left[b,i,h,k]*right[b,j,h,k]
    out[b,i,j,p] = sum_h out[b,i,j,h]*W_out[h,p] + b_out[p]   [B, L, L, P]

Mathematical restructuring (c = (h,k) combined channel, 0..31):
    out[b,i,j,p] = sum_c right[b,j,c] * (left[b,i,c] * W2[c,p]) + b_out[p]
with W2[c,p] = W_out[c//2, p].  M-packs M[c, (q,p)] = left[b,i_q,c]*W2[c,p]
for 4 i's are built in one broadcast multiply each; the pair matmul is
bf16 x bf16 -> fp32 PSUM:  lhsT=rightT[33, j-chunk 128] x rhs=M_pack[33, 512].
Row 32 of rightT is constant 1 and row 32 of the M-pack is b_out, which adds
the bias inside the same matmul.

PSUM is drained in [128, 1024] bank pairs with an f32->bf16 converting copy
(DVE/ACT alternating) into 1 MiB staging tiles, DMA'd per j-chunk.  The host
converts bf16 back to fp32 while assembling (bf16 + bf16-matmul rounding is
~6e-3 max-rel, inside the 2e-2 gate).

Sharding: the i axis of L is split across the 8 cores (sequence-parallel);
each core holds its [B, 64] slice of `left` plus the full `right` side and
writes a [B, 64, L, P] output shard.  No cross-device communication.

LayerNorm gamma/beta are folded into the projection weights on the host
(exact algebra): W_e = gamma[:,None]*W, with an extra K=1 accumulation row
carrying beta@W * mask.
"""

import os
import sys

sys.path.insert(0, "/opt/trn_rl_repo")

import numpy as np

import concourse.bass as bass
import concourse.mybir as mybir
import concourse.tile as tile
from concourse import bacc
from concourse.bass_utils import run_bass_kernel_spmd
from concourse.masks import make_identity

F32 = mybir.dt.float32
F32R = mybir.dt.float32r
BF16 = mybir.dt.bfloat16

B, L, D = 2, 512, 256
DH, H, PAIR = 32, 16, 128
NCORES = 8
LSH = L // NCORES          # 64 i's per core per batch
LN_EPS = 1e-5

# packed-constant column map (one [128, NCONST] f32 tensor, one DMA)
COL_WL = (0, 32)           # [128, 32] x2: gamma*W_l rows 0-127 / 128-255
COL_WR = (64, 96)
COL_W2 = 128               # [33, 128] (cast to bf16 on chip)
COL_MCF = 256              # [128, 8]
COL_MCS = 264              # [128, 1]
COL_BL = 265               # [32, 1]
COL_BR = 266               # [32, 1]
COL_WLR = 267              # [1, 32]  row 256 of w_left_e
COL_WRR = 299              # [1, 32]
COL_MRS = 331              # [1, 128]
COL_MRF = (459, 971)       # [1, 512] x2
NCONST = 1483

_COMPILED = None  # (nc, input_names)


def _build_program():
    nc = bacc.Bacc("TRN2", target_bir_lowering=False, debug=False,
                   num_devices=NCORES)

    node_full = nc.dram_tensor("node_full", [B * L, D], F32,
                               kind="ExternalInput").ap()
    node_shard = nc.dram_tensor("node_shard", [B * LSH, D], F32,
                                kind="ExternalInput").ap()
    consts = nc.dram_tensor("consts", [128, NCONST], F32,
                            kind="ExternalInput").ap()

    # Permuted output layout: [b, jc, q2, j, s, i16, p] (bf16) — each staging
    # buffer lands as one fully contiguous 1 MiB stream (8 KiB per partition
    # run).  sg = q2*2 + s; i_local = sg*16 + i16.  The host un-permutes +
    # upcasts while assembling the full output.
    out = nc.dram_tensor("out", [B, 4, 2, 128, 2, 16, PAIR], BF16,
                         kind="ExternalOutput").ap()

    with tile.TileContext(nc) as tc:
        with (
            tc.tile_pool(name="singles", bufs=1) as singles,
            tc.tile_pool(name="xpool", bufs=9) as xpool,
            tc.tile_pool(name="stats", bufs=4) as stats,
            tc.tile_pool(name="persist", bufs=1) as persist,
            tc.tile_pool(name="mp", bufs=8) as mp_pool,
            tc.tile_pool(name="stag", bufs=8) as stag_pool,
            tc.tile_pool(name="ps_tp", bufs=2, space="PSUM") as ps_tp,
            tc.tile_pool(name="ps_proj", bufs=1, space="PSUM") as ps_proj,
            tc.tile_pool(name="ps_big", bufs=2, space="PSUM") as ps_big,
        ):
            # ---------------- input loads (3 queues in parallel) ------------
            xs = xpool.tile([128, D], F32, tag="x", name="xs")
            nc.sync.dma_start(out=xs, in_=node_shard[:, :])
            cst = singles.tile([128, NCONST], F32, tag="cst")
            nc.scalar.dma_start(out=cst, in_=consts[:, :])
            xf_tiles = []
            qs = [nc.sync, nc.scalar, nc.gpsimd]
            for t in range(8):
                xf = xpool.tile([128, D], F32, tag="x", name=f"xf{t}")
                qs[[0, 0, 2, 2, 1, 1, 2, 0][t]].dma_start(
                    out=xf, in_=node_full[t * 128:(t + 1) * 128, :])
                xf_tiles.append(xf)

            # ---------------- constants / views ----------------
            ident = singles.tile([128, 128], F32, tag="ident")
            make_identity(nc, ident)
            eps_t = singles.tile([128, 1], F32, tag="eps")
            nc.vector.memset(eps_t, LN_EPS)

            wl_sb = [cst[:, COL_WL[dc]:COL_WL[dc] + DH] for dc in range(2)]
            wr_sb = [cst[:, COL_WR[dc]:COL_WR[dc] + DH] for dc in range(2)]
            wl_row = cst[0:1, COL_WLR:COL_WLR + DH]
            wr_row = cst[0:1, COL_WRR:COL_WRR + DH]
            bl_sb = cst[0:DH, COL_BL:COL_BL + 1]
            br_sb = cst[0:DH, COL_BR:COL_BR + 1]
            mcf_sb = cst[:, COL_MCF:COL_MCF + 8]
            mcs_sb = cst[:, COL_MCS:COL_MCS + 1]
            mrs_sb = cst[0:1, COL_MRS:COL_MRS + B * LSH]
            mrf_sb = [cst[0:1, COL_MRF[b]:COL_MRF[b] + L] for b in range(B)]

            w2bf = singles.tile([DH + 1, PAIR], BF16, tag="w2bf")
            nc.scalar.copy(out=w2bf, in_=cst[0:DH + 1, COL_W2:COL_W2 + PAIR])

            # ---------------- LayerNorm helper ----------------
            def layernorm_masked(x_t, mask_col_ap):
                """x_t [128, D] in place -> (x - mu) * rsqrt(var+eps) * mask."""
                st = stats.tile([128, 6], F32, tag="st")
                nc.vector.bn_stats(out=st, in_=x_t)
                mv = stats.tile([128, 2], F32, tag="mv")
                nc.vector.bn_aggr(out=mv, in_=st)
                sd = stats.tile([128, 1], F32, tag="sd")
                nc.scalar.activation(out=sd, in_=mv[:, 1:2],
                                     func=mybir.ActivationFunctionType.Sqrt,
                                     bias=eps_t, scale=1.0)
                rs = stats.tile([128, 1], F32, tag="rs")
                nc.vector.reciprocal(out=rs, in_=sd)
                rsm = stats.tile([128, 1], F32, tag="rsm")
                nc.vector.tensor_mul(out=rsm, in0=rs, in1=mask_col_ap)
                nc.vector.tensor_scalar(out=x_t, in0=x_t,
                                        scalar1=mv[:, 0:1], scalar2=rsm,
                                        op0=mybir.AluOpType.subtract,
                                        op1=mybir.AluOpType.mult)

            # ---------------- shard path: leftT_all [33, B*LSH] bf16 --------
            layernorm_masked(xs, mcs_sb)

            xsT = [persist.tile([128, B * LSH], F32, tag=f"xsT{dc}",
                                name=f"xsT{dc}") for dc in range(2)]
            for dc in range(2):
                pt = ps_tp.tile([128, 128], F32, tag="tp")
                nc.tensor.transpose(pt, xs[:, dc * 128:(dc + 1) * 128], ident)
                nc.scalar.copy(out=xsT[dc], in_=pt)

            ps_l = ps_proj.tile([DH, L], F32, tag="pr", name="ps_l")
            ps_l = ps_l[:, 0:B * LSH]
            for dc in range(2):
                nc.tensor.matmul(ps_l, wl_sb[dc], xsT[dc],
                                 start=(dc == 0), stop=False)
            nc.tensor.matmul(ps_l, wl_row, mrs_sb, start=False, stop=True)
            leftT = persist.tile([DH + 1, B * LSH], BF16, tag="leftT")
            nc.scalar.activation(out=leftT[0:DH, :], in_=ps_l,
                                 func=mybir.ActivationFunctionType.Identity,
                                 bias=bl_sb, scale=1.0)
            nc.vector.memset(leftT[DH:DH + 1, :], 1.0)

            # ---------------- full path (per batch): rightT[b] [33, L] bf16 -
            rightT = [persist.tile([DH + 1, L], BF16, tag=f"rt{b}",
                                   name=f"rt{b}") for b in range(B)]
            xT = [[persist.tile([128, L], F32, tag=f"xT{b}_{dc}",
                                name=f"xT{b}_{dc}") for dc in range(2)]
                  for b in range(B)]

            def build_right(b):
                ps_r = ps_proj.tile([DH, L], F32, tag="pr", name=f"ps_r{b}")
                for lc in range(4):
                    t = b * 4 + lc
                    xf = xf_tiles[t]
                    layernorm_masked(xf, mcf_sb[:, t:t + 1])
                    for dc in range(2):
                        pt = ps_tp.tile([128, 128], F32, tag="tp")
                        nc.tensor.transpose(pt, xf[:, dc * 128:(dc + 1) * 128],
                                            ident)
                        nc.scalar.copy(
                            out=xT[b][dc][:, lc * 128:(lc + 1) * 128],
                            in_=pt)
                    # project this j-chunk as soon as its columns exist
                    jc = lc
                    jsl = slice(jc * 128, (jc + 1) * 128)
                    for dc in range(2):
                        nc.tensor.matmul(ps_r[:, jsl], wr_sb[dc],
                                         xT[b][dc][:, jsl],
                                         start=(dc == 0), stop=False)
                    nc.tensor.matmul(ps_r[:, jsl], wr_row, mrf_sb[b][:, jsl],
                                     start=False, stop=True)
                    nc.scalar.activation(out=rightT[b][0:DH, jsl],
                                         in_=ps_r[:, jsl],
                                         func=mybir.ActivationFunctionType.Identity,
                                         bias=br_sb, scale=1.0)
                nc.vector.memset(rightT[b][DH:DH + 1, :], 1.0)

            # ---------------- main pair loop ----------------
            def build_mp8(b, sg, ilp):
                """[33, 1024] bf16 pack for 8 i's (il = 2*ilp, 2*ilp+1):
                mp[c, (il8, q)*128 + p] = left[b, i, c] * w2[c, p]."""
                mp = mp_pool.tile([DH + 1, 1024], BF16, tag="mp",
                                  name=f"mp{b}_{sg}_{ilp}")
                col = b * LSH + (sg * 4 + ilp * 2) * 4
                lsrc = leftT[:, col:col + 8].unsqueeze(-1).to_broadcast(
                    [DH + 1, 8, PAIR])
                wsrc = w2bf[:, :].unsqueeze(1).to_broadcast([DH + 1, 8, PAIR])
                dst = mp[:, :].rearrange("c (q p) -> c q p", p=PAIR)
                nc.gpsimd.tensor_mul(out=dst, in0=wsrc, in1=lsrc)
                return mp

            def pair_rounds(b):
                for q2 in range(2):
                    mps = [[build_mp8(b, q2 * 2 + s, ilp) for ilp in range(2)]
                           for s in range(2)]
                    for jc in range(4):
                        lhsT = rightT[b][:, jc * 128:(jc + 1) * 128]
                        stg = stag_pool.tile([128, 2 * 16 * PAIR], BF16,
                                             tag="stag")
                        for s in range(2):
                            for ilp in range(2):
                                pb = ps_big.tile([128, 1024], F32, tag="big")
                                nc.tensor.matmul(pb[:, 0:512], lhsT,
                                                 mps[s][ilp][:, 0:512],
                                                 start=True, stop=True)
                                nc.tensor.matmul(pb[:, 512:1024], lhsT,
                                                 mps[s][ilp][:, 512:1024],
                                                 start=True, stop=True)
                                dst = stg[:, (s * 16 + ilp * 8) * PAIR:
                                          (s * 16 + ilp * 8 + 8) * PAIR]
                                if (s + ilp) % 2 == 0:
                                    nc.vector.tensor_copy(out=dst, in_=pb)
                                else:
                                    nc.scalar.copy(out=dst, in_=pb)
                        dst_ap = out[b, jc, q2, :, :, :, :]
                        src_ap = stg[:, :].rearrange("j (s i p) -> j s i p",
                                                     s=2, p=PAIR)
                        nc.sync.dma_start(out=dst_ap, in_=src_ap)

            build_right(0)
            pair_rounds(0)
            build_right(1)
            pair_rounds(1)

    nc.compile()
    names = ["node_full", "node_shard", "consts"]
    return nc, names


def _prepare_in_maps(node, mask, ln_gamma, ln_beta, W_left, b_left, W_right,
                     b_right, W_out, b_out):
    f = np.float32
    node = np.ascontiguousarray(np.asarray(node, dtype=f))        # [B, L, D]
    mask_f = np.asarray(mask).astype(f)                           # [B, L]
    gamma = np.asarray(ln_gamma, dtype=f)
    beta = np.asarray(ln_beta, dtype=f)
    W_l = np.asarray(W_left, dtype=f)
    W_r = np.asarray(W_right, dtype=f)
    b_l = np.asarray(b_left, dtype=f)
    b_r = np.asarray(b_right, dtype=f)
    W_o = np.asarray(W_out, dtype=f)
    b_o = np.asarray(b_out, dtype=f)

    s = 1.0 / np.sqrt(np.float32(DH))
    w_left_e = np.concatenate([gamma[:, None] * W_l, (beta @ W_l)[None, :]], 0)
    w_right_e = np.concatenate([gamma[:, None] * W_r, (beta @ W_r)[None, :]],
                               0) * s
    w2 = np.concatenate([np.repeat(W_o, 2, axis=0), b_o[None, :]], 0)

    common_cst = np.zeros((128, NCONST), f)
    for dc in range(2):
        common_cst[:, COL_WL[dc]:COL_WL[dc] + DH] = \
            w_left_e[dc * 128:(dc + 1) * 128]
        common_cst[:, COL_WR[dc]:COL_WR[dc] + DH] = \
            w_right_e[dc * 128:(dc + 1) * 128]
    common_cst[0:DH + 1, COL_W2:COL_W2 + PAIR] = w2
    common_cst[:, COL_MCF:COL_MCF + 8] = mask_f.reshape(-1, 128).T
    common_cst[0:DH, COL_BL] = b_l
    common_cst[0:DH, COL_BR] = b_r * s
    common_cst[0, COL_WLR:COL_WLR + DH] = w_left_e[D]
    common_cst[0, COL_WRR:COL_WRR + DH] = w_right_e[D]
    for b in range(B):
        common_cst[0, COL_MRF[b]:COL_MRF[b] + L] = mask_f[b]

    node_flat = node.reshape(B * L, D)

    in_maps = []
    for c in range(NCORES):
        sl = slice(c * LSH, (c + 1) * LSH)
        shard = np.ascontiguousarray(node[:, sl, :].reshape(B * LSH, D))
        msk = mask_f[:, sl]                                       # [B, LSH]
        cstc = common_cst.copy()
        cstc[:, COL_MCS] = msk.reshape(-1)
        cstc[0, COL_MRS:COL_MRS + B * LSH] = msk.reshape(-1)
        in_maps.append({
            "node_full": node_flat,
            "node_shard": shard,
            "consts": cstc,
        })
    return in_maps


def kernel(**inputs):
    global _COMPILED
    if _COMPILED is None:
        _COMPILED = _build_program()
    nc, names = _COMPILED
    in_maps = _prepare_in_maps(**inputs)
    res = run_bass_kernel_spmd(nc, in_maps, core_ids=list(range(NCORES)))
    full = np.empty((B, L, L, PAIR), np.float32)
    for c in range(NCORES):
        dev = res.results[c]["out"]   # [b, jc, q2, j, s, i16, p] bf16
        full[:, c * LSH:(c + 1) * LSH] = (
            dev.transpose(0, 2, 4, 5, 1, 3, 6)
               .reshape(B, LSH, L, PAIR).astype(np.float32))
    return full


if __name__ == "__main__":
    # self-test with NON-trivial gamma/beta/mask against a numpy reference
    rng = np.random.default_rng(1)
    mask = np.ones((B, L), dtype=bool)
    mask[0, 500:] = False        # exercise the mask path
    mask[1, :3] = False
    inputs = {
        "node": rng.standard_normal((B, L, D)).astype(np.float32),
        "mask": mask,
        "ln_gamma": (1.0 + 0.1 * rng.standard_normal(D)).astype(np.float32),
        "ln_beta": (0.1 * rng.standard_normal(D)).astype(np.float32),
        "W_left": (rng.standard_normal((D, DH)) / np.sqrt(D)).astype(np.float32),
        "b_left": (0.1 * rng.standard_normal(DH)).astype(np.float32),
        "W_right": (rng.standard_normal((D, DH)) / np.sqrt(D)).astype(np.float32),
        "b_right": (0.1 * rng.standard_normal(DH)).astype(np.float32),
        "W_out": (rng.standard_normal((H, PAIR)) / np.sqrt(H)).astype(np.float32),
        "b_out": (0.1 * rng.standard_normal(PAIR)).astype(np.float32),
    }

    def np_reference(node, mask, ln_gamma, ln_beta, W_left, b_left, W_right,
                     b_right, W_out, b_out):
        node = node.astype(np.float64)
        mu = node.mean(-1, keepdims=True)
        var = ((node - mu) ** 2).mean(-1, keepdims=True)
        x = (node - mu) / np.sqrt(var + LN_EPS) * ln_gamma + ln_beta
        x = x * mask[..., None]
        left = (x @ W_left + b_left).reshape(B, L, H, -1)
        right = ((x @ W_right + b_right) / np.sqrt(DH)).reshape(B, L, H, -1)
        o = np.einsum("bihk,bjhk->bijh", left, right)
        return np.einsum("bijh,hp->bijp", o, W_out) + b_out

    got = kernel(**inputs)
    exp = np_reference(**inputs)
    rel = np.abs(got - exp).max() / np.abs(exp).max()
    print("general-path rel err:", rel)
    assert rel < 1.5e-2, rel
    print("OK", got.shape, got.dtype)


# revision 13
# speedup vs baseline: 1.1957x; 1.0653x over previous
"""Trainium2 Bass kernel for nn_Node2Pair_bias (LayerNorm -> dual projection ->
pair outer-product -> head-mix linear).

Reference computation (B=2, L=512, D=256, DH=32, H=16, K=2, P=128):
    x   = LayerNorm(node) * gamma + beta, masked        [B, L, D]
    left  = (x @ W_left + b_left)                       [B, L, DH] -> [B,L,H,K]
    right = (x @ W_right + b_right)/sqrt(DH)            [B, L, DH] -> [B,L,H,K]
    out[b,i,j,h] = sum_k left[b,i,h,k]*right[b,j,h,k]
    out[b,i,j,p] = sum_h out[b,i,j,h]*W_out[h,p] + b_out[p]   [B, L, L, P]

Mathematical restructuring (c = (h,k) combined channel, 0..31):
    out[b,i,j,p] = sum_c right[b,j,c] * (left[b,i,c] * W2[c,p]) + b_out[p]
with W2[c,p] = W_out[c//2, p].  M-packs M[c, (q,p)] = left[b,i_q,c]*W2[c,p]
for 4 i's are built in one broadcast multiply each; the pair matmul is
bf16 x bf16 -> fp32 PSUM:  lhsT=rightT[33, j-chunk 128] x rhs=M_pack[33, 512].
Row 32 of rightT is constant 1 and row 32 of the M-pack is b_out, which adds
the bias inside the same matmul.

PSUM is drained in [128, 1024] bank pairs with an f32->bf16 converting copy
(DVE/ACT alternating) into 1 MiB staging tiles, DMA'd per j-chunk.  The host
converts bf16 back to fp32 while assembling (bf16 + bf16-matmul rounding is
~6e-3 max-rel, inside the 2e-2 gate).

Sharding: the i axis of L is split across the 8 cores (sequence-parallel);
each core holds its [B, 64] slice of `left` plus the full `right` side and
writes a [B, 64, L, P] output shard.  No cross-device communication.

LayerNorm gamma/beta are folded into the projection weights on the host
(exact algebra): W_e = gamma[:,None]*W, with an extra K=1 accumulation row
carrying beta@W * mask.
"""

import os
import sys

sys.path.insert(0, "/opt/trn_rl_repo")

import numpy as np

import concourse.bass as bass
import concourse.mybir as mybir
import concourse.tile as tile
from concourse import bacc
from concourse.bass_utils import run_bass_kernel_spmd
from concourse.masks import make_identity

F32 = mybir.dt.float32
F32R = mybir.dt.float32r
BF16 = mybir.dt.bfloat16

B, L, D = 2, 512, 256
DH, H, PAIR = 32, 16, 128
NCORES = 8
LSH = L // NCORES          # 64 i's per core per batch
LN_EPS = 1e-5

# packed-constant column map (one [128, NCONST] f32 tensor, one DMA)
COL_WL = (0, 32)           # [128, 32] x2: gamma*W_l rows 0-127 / 128-255
COL_WR = (64, 96)
COL_W2 = 128               # [33, 128] (cast to bf16 on chip)
COL_MCF = 256              # [128, 8]
COL_MCS = 264              # [128, 1]
COL_BL = 265               # [32, 1]
COL_BR = 266               # [32, 1]
COL_WLR = 267              # [1, 32]  row 256 of w_left_e
COL_WRR = 299              # [1, 32]
COL_MRS = 331              # [1, 128]
COL_MRF = (459, 971)       # [1, 512] x2
NCONST = 1483

_COMPILED = None  # (nc, input_names)


def _build_program():
    nc = bacc.Bacc("TRN2", target_bir_lowering=False, debug=False,
                   num_devices=NCORES)

    node_full = nc.dram_tensor("node_full", [B * L, D], F32,
                               kind="ExternalInput").ap()
    node_shard = nc.dram_tensor("node_shard", [B * LSH, D], F32,
                                kind="ExternalInput").ap()
    consts = nc.dram_tensor("consts", [128, NCONST], F32,
                            kind="ExternalInput").ap()

    # Permuted output layout: [b, jc, q2, j, s, i16, p] (bf16) — each staging
    # buffer lands as one fully contiguous 1 MiB stream (8 KiB per partition
    # run).  sg = q2*2 + s; i_local = sg*16 + i16.  The host un-permutes +
    # upcasts while assembling the full output.
    out = nc.dram_tensor("out", [B, 4, 2, 128, 2, 16, PAIR], BF16,
                         kind="ExternalOutput").ap()

    with tile.TileContext(nc) as tc:
        with (
            tc.tile_pool(name="singles", bufs=1) as singles,
            tc.tile_pool(name="xpool", bufs=9) as xpool,
            tc.tile_pool(name="stats", bufs=4) as stats,
            tc.tile_pool(name="persist", bufs=1) as persist,
            tc.tile_pool(name="mp", bufs=8) as mp_pool,
            tc.tile_pool(name="stag", bufs=8) as stag_pool,
            tc.tile_pool(name="ps_proj", bufs=1, space="PSUM") as ps_proj,
            tc.tile_pool(name="ps_big", bufs=3, space="PSUM") as ps_big,
        ):
            # ---------------- input loads (3 queues in parallel) ------------
            xs = xpool.tile([128, D], F32, tag="x", name="xs")
            nc.sync.dma_start(out=xs, in_=node_shard[:, :])
            cst = singles.tile([128, NCONST], F32, tag="cst")
            nc.scalar.dma_start(out=cst, in_=consts[:, :])
            xf_tiles = []
            qs = [nc.sync, nc.scalar, nc.gpsimd]
            for t in range(8):
                xf = xpool.tile([128, D], F32, tag="x", name=f"xf{t}")
                qs[[0, 0, 2, 2, 1, 1, 2, 0][t]].dma_start(
                    out=xf, in_=node_full[t * 128:(t + 1) * 128, :])
                xf_tiles.append(xf)

            # ---------------- constants / views ----------------
            ident = singles.tile([128, 128], F32, tag="ident")
            make_identity(nc, ident)
            eps_t = singles.tile([128, 1], F32, tag="eps")
            nc.vector.memset(eps_t, LN_EPS)

            wl_sb = [cst[:, COL_WL[dc]:COL_WL[dc] + DH] for dc in range(2)]
            wr_sb = [cst[:, COL_WR[dc]:COL_WR[dc] + DH] for dc in range(2)]
            wl_row = cst[0:1, COL_WLR:COL_WLR + DH]
            wr_row = cst[0:1, COL_WRR:COL_WRR + DH]
            bl_sb = cst[0:DH, COL_BL:COL_BL + 1]
            br_sb = cst[0:DH, COL_BR:COL_BR + 1]
            mcf_sb = cst[:, COL_MCF:COL_MCF + 8]
            mcs_sb = cst[:, COL_MCS:COL_MCS + 1]
            mrs_sb = cst[0:1, COL_MRS:COL_MRS + B * LSH]
            mrf_sb = [cst[0:1, COL_MRF[b]:COL_MRF[b] + L] for b in range(B)]

            w2bf = singles.tile([DH + 1, PAIR], BF16, tag="w2bf")
            nc.scalar.copy(out=w2bf, in_=cst[0:DH + 1, COL_W2:COL_W2 + PAIR])

            # ---------------- LayerNorm helper ----------------
            def layernorm_masked(x_t, mask_col_ap):
                """x_t [128, D] in place -> (x - mu) * rsqrt(var+eps) * mask."""
                st = stats.tile([128, 6], F32, tag="st")
                nc.vector.bn_stats(out=st, in_=x_t)
                mv = stats.tile([128, 2], F32, tag="mv")
                nc.vector.bn_aggr(out=mv, in_=st)
                sd = stats.tile([128, 1], F32, tag="sd")
                nc.scalar.activation(out=sd, in_=mv[:, 1:2],
                                     func=mybir.ActivationFunctionType.Sqrt,
                                     bias=eps_t, scale=1.0)
                rs = stats.tile([128, 1], F32, tag="rs")
                nc.vector.reciprocal(out=rs, in_=sd)
                rsm = stats.tile([128, 1], F32, tag="rsm")
                nc.vector.tensor_mul(out=rsm, in0=rs, in1=mask_col_ap)
                nc.vector.tensor_scalar(out=x_t, in0=x_t,
                                        scalar1=mv[:, 0:1], scalar2=rsm,
                                        op0=mybir.AluOpType.subtract,
                                        op1=mybir.AluOpType.mult)

            # ---------------- shard path: leftT_all [33, B*LSH] bf16 --------
            layernorm_masked(xs, mcs_sb)

            xsT = [persist.tile([128, B * LSH], F32, tag=f"xsT{dc}",
                                name=f"xsT{dc}") for dc in range(2)]
            for dc in range(2):
                pt = ps_big.tile([128, 1024], F32, tag="big",
                                 name=f"tps{dc}")[:, 0:128]
                nc.tensor.transpose(pt, xs[:, dc * 128:(dc + 1) * 128], ident)
                nc.scalar.copy(out=xsT[dc], in_=pt)

            ps_l = ps_proj.tile([DH, L], F32, tag="pr", name="ps_l")
            ps_l = ps_l[:, 0:B * LSH]
            for dc in range(2):
                nc.tensor.matmul(ps_l, wl_sb[dc], xsT[dc],
                                 start=(dc == 0), stop=False)
            nc.tensor.matmul(ps_l, wl_row, mrs_sb, start=False, stop=True)
            leftT = persist.tile([DH + 1, B * LSH], BF16, tag="leftT")
            nc.scalar.activation(out=leftT[0:DH, :], in_=ps_l,
                                 func=mybir.ActivationFunctionType.Identity,
                                 bias=bl_sb, scale=1.0)
            nc.vector.memset(leftT[DH:DH + 1, :], 1.0)

            # ---------------- full path (per batch): rightT[b] [33, L] bf16 -
            rightT = [persist.tile([DH + 1, L], BF16, tag=f"rt{b}",
                                   name=f"rt{b}") for b in range(B)]
            xT = [[persist.tile([128, L], F32, tag=f"xT{b}_{dc}",
                                name=f"xT{b}_{dc}") for dc in range(2)]
                  for b in range(B)]

            def build_right(b):
                ps_r = ps_proj.tile([DH, L], F32, tag="pr", name=f"ps_r{b}")
                for lc in range(4):
                    t = b * 4 + lc
                    xf = xf_tiles[t]
                    layernorm_masked(xf, mcf_sb[:, t:t + 1])
                    for dc in range(2):
                        pt = ps_big.tile([128, 1024], F32, tag="big",
                                         name=f"tp{b}_{lc}_{dc}")[:, 0:128]
                        nc.tensor.transpose(pt, xf[:, dc * 128:(dc + 1) * 128],
                                            ident)
                        nc.scalar.copy(
                            out=xT[b][dc][:, lc * 128:(lc + 1) * 128],
                            in_=pt)
                    # project this j-chunk as soon as its columns exist
                    jc = lc
                    jsl = slice(jc * 128, (jc + 1) * 128)
                    for dc in range(2):
                        nc.tensor.matmul(ps_r[:, jsl], wr_sb[dc],
                                         xT[b][dc][:, jsl],
                                         start=(dc == 0), stop=False)
                    nc.tensor.matmul(ps_r[:, jsl], wr_row, mrf_sb[b][:, jsl],
                                     start=False, stop=True)
                    nc.scalar.activation(out=rightT[b][0:DH, jsl],
                                         in_=ps_r[:, jsl],
                                         func=mybir.ActivationFunctionType.Identity,
                                         bias=br_sb, scale=1.0)
                nc.vector.memset(rightT[b][DH:DH + 1, :], 1.0)

            # ---------------- main pair loop ----------------
            def build_mp8(b, sg, ilp):
                """[33, 1024] bf16 pack for 8 i's (il = 2*ilp, 2*ilp+1):
                mp[c, (il8, q)*128 + p] = left[b, i, c] * w2[c, p]."""
                mp = mp_pool.tile([DH + 1, 1024], BF16, tag="mp",
                                  name=f"mp{b}_{sg}_{ilp}")
                col = b * LSH + (sg * 4 + ilp * 2) * 4
                lsrc = leftT[:, col:col + 8].unsqueeze(-1).to_broadcast(
                    [DH + 1, 8, PAIR])
                wsrc = w2bf[:, :].unsqueeze(1).to_broadcast([DH + 1, 8, PAIR])
                dst = mp[:, :].rearrange("c (q p) -> c q p", p=PAIR)
                nc.gpsimd.tensor_mul(out=dst, in0=wsrc, in1=lsrc)
                return mp

            def pair_rounds(b):
                for q2 in range(2):
                    mps = [[build_mp8(b, q2 * 2 + s, ilp) for ilp in range(2)]
                           for s in range(2)]
                    for jc in range(4):
                        lhsT = rightT[b][:, jc * 128:(jc + 1) * 128]
                        stg = stag_pool.tile([128, 2 * 16 * PAIR], BF16,
                                             tag="stag")
                        for s in range(2):
                            for ilp in range(2):
                                pb = ps_big.tile([128, 1024], F32, tag="big")
                                nc.tensor.matmul(pb[:, 0:512], lhsT,
                                                 mps[s][ilp][:, 0:512],
                                                 start=True, stop=True)
                                nc.tensor.matmul(pb[:, 512:1024], lhsT,
                                                 mps[s][ilp][:, 512:1024],
                                                 start=True, stop=True)
                                dst = stg[:, (s * 16 + ilp * 8) * PAIR:
                                          (s * 16 + ilp * 8 + 8) * PAIR]
                                if (s + ilp) % 2 == 0:
                                    nc.vector.tensor_copy(out=dst, in_=pb)
                                else:
                                    nc.scalar.copy(out=dst, in_=pb)
                        dst_ap = out[b, jc, q2, :, :, :, :]
                        src_ap = stg[:, :].rearrange("j (s i p) -> j s i p",
                                                     s=2, p=PAIR)
                        deng = nc.sync if jc % 2 == 0 else nc.gpsimd
                        deng.dma_start(out=dst_ap, in_=src_ap)

            build_right(0)
            pair_rounds(0)
            build_right(1)
            pair_rounds(1)

    nc.compile()
    names = ["node_full", "node_shard", "consts"]
    return nc, names


def _prepare_in_maps(node, mask, ln_gamma, ln_beta, W_left, b_left, W_right,
                     b_right, W_out, b_out):
    f = np.float32
    node = np.ascontiguousarray(np.asarray(node, dtype=f))        # [B, L, D]
    mask_f = np.asarray(mask).astype(f)                           # [B, L]
    gamma = np.asarray(ln_gamma, dtype=f)
    beta = np.asarray(ln_beta, dtype=f)
    W_l = np.asarray(W_left, dtype=f)
    W_r = np.asarray(W_right, dtype=f)
    b_l = np.asarray(b_left, dtype=f)
    b_r = np.asarray(b_right, dtype=f)
    W_o = np.asarray(W_out, dtype=f)
    b_o = np.asarray(b_out, dtype=f)

    s = 1.0 / np.sqrt(np.float32(DH))
    w_left_e = np.concatenate([gamma[:, None] * W_l, (beta @ W_l)[None, :]], 0)
    w_right_e = np.concatenate([gamma[:, None] * W_r, (beta @ W_r)[None, :]],
                               0) * s
    w2 = np.concatenate([np.repeat(W_o, 2, axis=0), b_o[None, :]], 0)

    common_cst = np.zeros((128, NCONST), f)
    for dc in range(2):
        common_cst[:, COL_WL[dc]:COL_WL[dc] + DH] = \
            w_left_e[dc * 128:(dc + 1) * 128]
        common_cst[:, COL_WR[dc]:COL_WR[dc] + DH] = \
            w_right_e[dc * 128:(dc + 1) * 128]
    common_cst[0:DH + 1, COL_W2:COL_W2 + PAIR] = w2
    common_cst[:, COL_MCF:COL_MCF + 8] = mask_f.reshape(-1, 128).T
    common_cst[0:DH, COL_BL] = b_l
    common_cst[0:DH, COL_BR] = b_r * s
    common_cst[0, COL_WLR:COL_WLR + DH] = w_left_e[D]
    common_cst[0, COL_WRR:COL_WRR + DH] = w_right_e[D]
    for b in range(B):
        common_cst[0, COL_MRF[b]:COL_MRF[b] + L] = mask_f[b]

    node_flat = node.reshape(B * L, D)

    in_maps = []
    for c in range(NCORES):
        sl = slice(c * LSH, (c + 1) * LSH)
        shard = np.ascontiguousarray(node[:, sl, :].reshape(B * LSH, D))
        msk = mask_f[:, sl]                                       # [B, LSH]
        cstc = common_cst.copy()
        cstc[:, COL_MCS] = msk.reshape(-1)
        cstc[0, COL_MRS:COL_MRS + B * LSH] = msk.reshape(-1)
        in_maps.append({
            "node_full": node_flat,
            "node_shard": shard,
            "consts": cstc,
        })
    return in_maps


def kernel(**inputs):
    global _COMPILED
    if _COMPILED is None:
        _COMPILED = _build_program()
    nc, names = _COMPILED
    in_maps = _prepare_in_maps(**inputs)
    res = run_bass_kernel_spmd(nc, in_maps, core_ids=list(range(NCORES)))
    full = np.empty((B, L, L, PAIR), np.float32)
    for c in range(NCORES):
        dev = res.results[c]["out"]   # [b, jc, q2, j, s, i16, p] bf16
        full[:, c * LSH:(c + 1) * LSH] = (
            dev.transpose(0, 2, 4, 5, 1, 3, 6)
               .reshape(B, LSH, L, PAIR).astype(np.float32))
    return full


if __name__ == "__main__":
    # self-test with NON-trivial gamma/beta/mask against a numpy reference
    rng = np.random.default_rng(1)
    mask = np.ones((B, L), dtype=bool)
    mask[0, 500:] = False        # exercise the mask path
    mask[1, :3] = False
    inputs = {
        "node": rng.standard_normal((B, L, D)).astype(np.float32),
        "mask": mask,
        "ln_gamma": (1.0 + 0.1 * rng.standard_normal(D)).astype(np.float32),
        "ln_beta": (0.1 * rng.standard_normal(D)).astype(np.float32),
        "W_left": (rng.standard_normal((D, DH)) / np.sqrt(D)).astype(np.float32),
        "b_left": (0.1 * rng.standard_normal(DH)).astype(np.float32),
        "W_right": (rng.standard_normal((D, DH)) / np.sqrt(D)).astype(np.float32),
        "b_right": (0.1 * rng.standard_normal(DH)).astype(np.float32),
        "W_out": (rng.standard_normal((H, PAIR)) / np.sqrt(H)).astype(np.float32),
        "b_out": (0.1 * rng.standard_normal(PAIR)).astype(np.float32),
    }

    def np_reference(node, mask, ln_gamma, ln_beta, W_left, b_left, W_right,
                     b_right, W_out, b_out):
        node = node.astype(np.float64)
        mu = node.mean(-1, keepdims=True)
        var = ((node - mu) ** 2).mean(-1, keepdims=True)
        x = (node - mu) / np.sqrt(var + LN_EPS) * ln_gamma + ln_beta
        x = x * mask[..., None]
        left = (x @ W_left + b_left).reshape(B, L, H, -1)
        right = ((x @ W_right + b_right) / np.sqrt(DH)).reshape(B, L, H, -1)
        o = np.einsum("bihk,bjhk->bijh", left, right)
        return np.einsum("bijh,hp->bijp", o, W_out) + b_out

    got = kernel(**inputs)
    exp = np_reference(**inputs)
    rel = np.abs(got - exp).max() / np.abs(exp).max()
    print("general-path rel err:", rel)
    assert rel < 1.5e-2, rel
    print("OK", got.shape, got.dtype)
